# revision 8
# baseline (speedup 1.0000x reference)
import sys

sys.path.insert(0, "/opt/trn_rl_repo")

import numpy as np

NUM_T = 1024
B, NC, NT = 32, 2048, 1024
NB = 4          # batches per core
NCORES = 8
ENC_JT = 16     # j-tiles of 128 sorted xc points
DEC_G = 8       # xt chunks of 128 sorted xt points

_CACHE = {}


def _build(key, repeat=1):
    (scale_psi, lnos_psi, scale_rho, lnos_rho, b4_0, b4_1,
     enc_W, enc_offs, dec_bands) = key
    from concourse import bass, bacc, tile, mybir

    F32 = mybir.dt.float32
    F32R = mybir.dt.float32r
    AF = mybir.ActivationFunctionType

    nc = bacc.Bacc("TRN2", target_bir_lowering=False, debug=False,
                   enable_asserts=False, num_devices=1)

    di = {}
    for name, shape in [
        ("xpoly", (3, NB * NC)), ("tpoly", (3, NUM_T)), ("tpoly2", (3, NUM_T)),
        ("xtpoly", (3, NB * NT)), ("trow", (1, NUM_T)),
        ("phip", (128, NB * ENC_JT * 33)),
        ("wt1", (3, 5 * 16)), ("wt2", (16, 5 * 32)), ("wt3", (32, 5 * 16)),
        ("wt4", (16, 5 * 33)),
        ("b1v", (16, 1)), ("b2v", (32, 1)), ("b3v", (16, 1)),
    ]:
        di[name] = nc.dram_tensor(name, shape, F32, kind="ExternalInput")
    d_mu = nc.dram_tensor("mu_out", (NB, NT), F32, kind="ExternalOutput")
    d_sg = nc.dram_tensor("sig_out", (NB, NT), F32, kind="ExternalOutput")

    with tile.TileContext(nc) as tc:
        with (
            tc.tile_pool(name="cst", bufs=1) as cst,
            tc.tile_pool(name="wrk", bufs=1) as wrk,
            tc.tile_pool(name="kep", bufs=2) as kep,
            tc.tile_pool(name="psw", bufs=2, space="PSUM") as psw,
            tc.tile_pool(name="psh", bufs=1, space="PSUM") as psh,
            tc.tile_pool(name="psm", bufs=1, space="PSUM") as psm,
        ):
            # ---- persistent constants ----
            c_xpoly = cst.tile([3, NB * NC], F32)
            nc.sync.dma_start(c_xpoly[:], di["xpoly"].ap())
            c_tpoly = cst.tile([3, NUM_T], F32)
            nc.sync.dma_start(c_tpoly[:], di["tpoly"].ap())
            c_tpoly2 = cst.tile([3, NUM_T], F32)
            nc.sync.dma_start(c_tpoly2[:], di["tpoly2"].ap())
            c_xtpoly = cst.tile([3, NB * NT], F32)
            nc.sync.dma_start(c_xtpoly[:], di["xtpoly"].ap())
            c_trow = cst.tile([1, NUM_T], F32)
            nc.sync.dma_start(c_trow[:], di["trow"].ap())
            c_phip = cst.tile([128, NB * ENC_JT * 33], F32R)
            nc.gpsimd.dma_start(c_phip[:], di["phip"].ap())
            c_wt1 = cst.tile([3, 80], F32R); nc.gpsimd.dma_start(c_wt1[:], di["wt1"].ap())
            c_wt2 = cst.tile([16, 160], F32R); nc.gpsimd.dma_start(c_wt2[:], di["wt2"].ap())
            c_wt3 = cst.tile([32, 80], F32R); nc.gpsimd.dma_start(c_wt3[:], di["wt3"].ap())
            c_wt4 = cst.tile([16, 165], F32R); nc.gpsimd.dma_start(c_wt4[:], di["wt4"].ap())
            c_b1 = cst.tile([16, 1], F32); nc.sync.dma_start(c_b1[:], di["b1v"].ap())
            c_b2 = cst.tile([32, 1], F32); nc.sync.dma_start(c_b2[:], di["b2v"].ap())
            c_b3 = cst.tile([16, 1], F32); nc.sync.dma_start(c_b3[:], di["b3v"].ap())

            c_bpsi = cst.tile([128, 1], F32); nc.vector.memset(c_bpsi[:], lnos_psi)
            c_brho = cst.tile([128, 1], F32); nc.vector.memset(c_brho[:], lnos_rho)
            c_b41 = cst.tile([33, 1], F32); nc.vector.memset(c_b41[:], b4_1)
            c_one = cst.tile([33, 1], F32); nc.vector.memset(c_one[:], 1.0)
            z_l33 = cst.tile([1, 33], F32R); nc.vector.memset(z_l33[:].bitcast(F32), 0.0)
            z_r = cst.tile([1, 512], F32R); nc.vector.memset(z_r[:].bitcast(F32), 0.0)

            def _body():
              for b in range(NB):
                # ================= encoder =================
                ph = psh.tile([33, NUM_T], F32, name="ph", tag="ph")
                # zero-fill ph with has_written set, via zero matmuls
                nc.tensor.matmul(ph[:, 0:512], z_l33[:], z_r[:], start=True, stop=False)
                nc.tensor.matmul(ph[:, 512:1024], z_l33[:], z_r[:], start=True, stop=False)

                # last h-mm piece per bank, to set stop flags
                pieces_all = []
                for jt in range(ENC_JT):
                    o = enc_offs[jt]
                    ps = []
                    if o < 512:
                        ps.append((o, min(o + enc_W, 512)))
                    if o + enc_W > 512:
                        ps.append((max(o, 512), o + enc_W))
                    pieces_all.append(ps)
                last_in_bank = {0: None, 1: None}
                for jt in range(ENC_JT):
                    for pi, (lo, hi) in enumerate(pieces_all[jt]):
                        last_in_bank[0 if lo < 512 else 1] = (jt, pi)

                GRP = 3  # j-tiles per psum tile
                for g0 in range(0, ENC_JT, GRP):
                    jts = list(range(g0, min(g0 + GRP, ENC_JT)))
                    pd = psw.tile([128, NUM_T], F32, name="pd", tag="pw")
                    for idx, jt in enumerate(jts):
                        o = enc_offs[jt]
                        s0 = enc_W * idx
                        # split the output slot at psum bank (512) boundaries
                        cuts = [s0, s0 + enc_W]
                        for edge in (512, 1024):
                            if s0 < edge < s0 + enc_W:
                                cuts.insert(-1, edge)
                        for lo, hi in zip(cuts[:-1], cuts[1:]):
                            nc.tensor.matmul(
                                pd[:, lo:hi],
                                c_xpoly[:, b * NC + jt * 128: b * NC + jt * 128 + 128],
                                c_tpoly[:, o + (lo - s0): o + (hi - s0)],
                                start=True, stop=True)
                    ke = kep.tile([128, NUM_T], F32R, name="ke", tag="ke")
                    nc.scalar.activation(
                        ke[:, 0: enc_W * len(jts)], pd[:, 0: enc_W * len(jts)],
                        AF.Exp, bias=c_bpsi[:], scale=scale_psi)
                    for idx, jt in enumerate(jts):
                        o = enc_offs[jt]
                        pc = (b * ENC_JT + jt) * 33
                        for pi, (lo, hi) in enumerate(pieces_all[jt]):
                            stop = (jt, pi) == last_in_bank[0 if lo < 512 else 1]
                            nc.tensor.matmul(
                                ph[:, lo:hi],
                                c_phip[:, pc: pc + 33],
                                ke[:, enc_W * idx + (lo - o): enc_W * idx + (hi - o)],
                                start=False, stop=stop)

                # ---- h features -> conv input X1 ----
                hs0 = wrk.tile([1, NUM_T], F32, name="hs0", tag="hs0")
                nc.vector.tensor_copy(hs0[:], ph[0:1, :])
                hs1 = wrk.tile([33, NUM_T], F32, name="hs1", tag="hs1")
                nc.vector.tensor_copy(hs1[32:33, :], ph[32:33, :])
                tA = wrk.tile([1, NUM_T], F32, name="tA", tag="tA")
                nc.vector.tensor_scalar_add(tA[:], hs0[:], 1e-8)
                tR = wrk.tile([1, NUM_T], F32, name="tR", tag="tR")
                nc.vector.reciprocal(tR[:], tA[:])
                tR32 = wrk.tile([33, NUM_T], F32, name="tR32", tag="tR32")
                nc.sync.dma_start(tR32[32:33, :], tR[:])
                tM = wrk.tile([33, NUM_T], F32, name="tM", tag="tM")
                nc.vector.tensor_mul(tM[32:33, :], hs1[32:33, :], tR32[32:33, :])

                X1 = wrk.tile([3, 1028], F32R, name="X1", tag="X1")
                nc.vector.memset(X1[:].bitcast(F32), 0.0)
                nc.gpsimd.dma_start(X1[0:1, 2:1026], c_trow[:])
                nc.gpsimd.dma_start(X1[1:2, 2:1026], hs0[:])
                nc.gpsimd.dma_start(X1[2:3, 2:1026], tM[32:33, :])

                # ================= conv tower =================
                def conv_layer(xin, wt, ctile, otile, bias_ap, oname):
                    pc = psw.tile([max(otile, 33), NUM_T], F32, name="pc" + oname, tag="pw")
                    for ch in (0, 512):
                        for dt in range(5):
                            nc.tensor.matmul(
                                pc[0:otile, ch: ch + 512],
                                wt[:, dt * otile: (dt + 1) * otile],
                                xin[:, dt + ch: dt + ch + 512],
                                start=(dt == 0), stop=(dt == 4))
                    xo = wrk.tile([otile, 1028], F32R, name=oname, tag=oname)
                    nc.vector.memset(xo[:].bitcast(F32), 0.0)
                    nc.scalar.activation(xo[:, 2:1026], pc[0:otile, :], AF.Relu,
                                         bias=bias_ap, scale=1.0)
                    return xo

                X2 = conv_layer(X1, c_wt1, 3, 16, c_b1[:], "X2")
                X3 = conv_layer(X2, c_wt2, 16, 32, c_b2[:], "X3")
                X4 = conv_layer(X3, c_wt3, 32, 16, c_b3[:], "X4")

                p4 = psw.tile([33, NUM_T], F32, name="p4", tag="pw")
                for ch in (0, 512):
                    for dt in range(5):
                        nc.tensor.matmul(
                            p4[:, ch: ch + 512],
                            c_wt4[:, dt * 33: (dt + 1) * 33],
                            X4[:, dt + ch: dt + ch + 512],
                            start=(dt == 0), stop=(dt == 4))

                S0 = wrk.tile([1, NUM_T], F32, name="S0", tag="S0")
                nc.scalar.activation(S0[:], p4[0:1, :], AF.Copy, bias=b4_0, scale=1.0)
                E32 = wrk.tile([33, NUM_T], F32, name="E32", tag="E32")
                nc.scalar.activation(E32[32:33, :], p4[32:33, :], AF.Exp,
                                     bias=c_b41[32:33, :], scale=1.0)
                SP = wrk.tile([33, NUM_T], F32, name="SP", tag="SP")
                nc.scalar.activation(SP[32:33, :], E32[32:33, :], AF.Ln,
                                     bias=c_one[32:33, :], scale=1.0)

                FSC = wrk.tile([32, NUM_T], F32, name="FSC", tag="FSC")
                nc.vector.memset(FSC[:], 0.0)
                nc.sync.dma_start(FSC[0:1, :], S0[:])
                nc.sync.dma_start(FSC[1:2, :], SP[32:33, :])
                FT32 = wrk.tile([128, 256], F32, name="FT32", tag="FT32")
                for k in range(8):
                    for m in range(4):
                        nc.vector.transpose(
                            FT32[32 * m: 32 * m + 32, 32 * k: 32 * k + 32],
                            FSC[:, 128 * k + 32 * m: 128 * k + 32 * m + 32])
                FDW = wrk.tile([128, 256], F32R, name="FDW", tag="FDW")
                nc.gpsimd.dma_start(FDW[:], FT32[:])

                # ================= decoder =================
                pmu = psm.tile([2, NUM_T], F32, name="pmu", tag="pmu")
                for g in range(DEC_G):
                    u0, kk = dec_bands[g]
                    pd2 = psw.tile([128, NUM_T], F32, name="pd2", tag="pw")
                    for s in range(kk):
                        nc.tensor.matmul(
                            pd2[:, 128 * s: 128 * s + 128],
                            c_tpoly2[:, 128 * (u0 + s): 128 * (u0 + s) + 128],
                            c_xtpoly[:, b * NT + 128 * g: b * NT + 128 * g + 128],
                            start=True, stop=True)
                    k2 = kep.tile([128, 512], F32R, name="k2", tag="k2")
                    nc.scalar.activation(k2[:, 0: 128 * kk], pd2[:, 0: 128 * kk],
                                         AF.Exp, bias=c_brho[:], scale=scale_rho)
                    for s in range(kk):
                        nc.tensor.matmul(
                            pmu[:, 128 * g: 128 * g + 128],
                            FDW[:, 32 * (u0 + s): 32 * (u0 + s) + 2],
                            k2[:, 128 * s: 128 * s + 128],
                            start=(s == 0), stop=(s == kk - 1))

                MU = wrk.tile([2, NUM_T], F32, name="MU", tag="MU")
                nc.vector.tensor_copy(MU[:], pmu[:])
                nc.sync.dma_start(d_mu.ap()[b: b + 1, :], MU[0:1, :])
                nc.sync.dma_start(d_sg.ap()[b: b + 1, :], MU[1:2, :])

            if repeat > 1:
                with tc.For_i(0, repeat, 1):
                    _body()
            else:
                _body()

    nc.compile()
    return nc


def kernel(xc, yc, xt, ls_psi, os_psi, ls_rho, os_rho,
           W1, b1, W2, b2, W3, b3, W4, b4):
    from concourse import bass_utils

    xc = np.asarray(xc, np.float32); yc = np.asarray(yc, np.float32)
    xt = np.asarray(xt, np.float32)
    ls_p = float(np.asarray(ls_psi).reshape(-1)[0])
    os_p = float(np.asarray(os_psi).reshape(-1)[0])
    ls_r = float(np.asarray(ls_rho).reshape(-1)[0])
    os_r = float(np.asarray(os_rho).reshape(-1)[0])

    xcf = xc[..., 0]; ycf = yc[..., 0]; xtf = xt[..., 0]
    lower = min(xcf.min(), xtf.min())
    upper = max(xcf.max(), xtf.max())
    t = np.linspace(lower, upper, NUM_T, dtype=np.float64).astype(np.float32)
    step = (upper - lower) / (NUM_T - 1)

    # sort per batch
    ixc = np.argsort(xcf, axis=1)
    xs = np.take_along_axis(xcf, ixc, axis=1)
    ys = np.take_along_axis(ycf, ixc, axis=1)
    ixt = np.argsort(xtf, axis=1)
    xts = np.take_along_axis(xtf, ixt, axis=1)

    # ---- global static windows (shared across cores) ----
    tau_e = 7.3 * ls_p
    tau_d = 7.3 * ls_r
    enc_offs = []
    enc_W = 288
    spans = []
    for jt in range(ENC_JT):
        lo = xs[:, jt * 128].min() - tau_e
        hi = xs[:, jt * 128 + 127].max() + tau_e
        spans.append(hi - lo)
    wmax = int(np.ceil(max(spans) / step)) + 4
    enc_W = max(288, ((wmax + 31) // 32) * 32)
    enc_W = min(enc_W, 1024)
    for jt in range(ENC_JT):
        lo = xs[:, jt * 128].min() - tau_e
        hi = xs[:, jt * 128 + 127].max() + tau_e
        mid = 0.5 * (lo + hi)
        o = int(round((mid - lower) / step)) - enc_W // 2
        o = max(0, min(NUM_T - enc_W, o))
        o = (o // 2) * 2
        # verify coverage
        assert o * step + lower <= lo + 1e-4 or o == 0
        assert (o + enc_W - 1) * step + lower >= hi - 1e-4 or o == NUM_T - enc_W
        enc_offs.append(o)

    dec_bands = []
    for g in range(DEC_G):
        lo = xts[:, g * 128].min() - tau_d
        hi = xts[:, g * 128 + 127].max() + tau_d
        u0 = int(np.floor((lo - lower) / step / 128.0))
        u1 = int(np.ceil((hi - lower) / step / 128.0))
        u0 = max(0, u0); u1 = min(8, u1); u1 = max(u1, u0 + 1)
        if u1 - u0 > 4:  # cap at 4 slabs (one psum bank)
            u0, u1 = u0, u0 + 4
        dec_bands.append((u0, u1 - u0))

    key = (float(-0.5 / (ls_p * ls_p)), float(np.log(os_p)),
           float(-0.5 / (ls_r * ls_r)), float(np.log(os_r)),
           float(np.asarray(b4).reshape(-1)[0]), float(np.asarray(b4).reshape(-1)[1]),
           enc_W, tuple(enc_offs), tuple(dec_bands))
    if key not in _CACHE:
        _CACHE[key] = _build(key)
    nc = _CACHE[key]
    global _LAST_KEY
    _LAST_KEY = key

    # ---- per-core input maps ----
    ones_t = np.ones_like(t)
    tpoly = np.stack([ones_t, -2.0 * t, t * t]).astype(np.float32)
    tpoly2 = np.stack([t * t, t, ones_t]).astype(np.float32)
    trow = t.reshape(1, -1)
    wt1 = np.transpose(np.asarray(W1, np.float32), (1, 2, 0)).reshape(3, 5 * 16, order="F")
    # careful: want wt[c, dt*O + o] = W[o, c, dt]
    def pack_w(W, O, C):
        out = np.zeros((C, 5 * O), np.float32)
        for dt in range(5):
            out[:, dt * O: (dt + 1) * O] = np.asarray(W, np.float32)[:, :, dt].T
        return out
    wt1 = pack_w(W1, 16, 3)
    wt2 = pack_w(W2, 32, 16)
    wt3 = pack_w(W3, 16, 32)
    wt4z = np.zeros((16, 5 * 33), np.float32)
    W4a = np.asarray(W4, np.float32)
    for dt in range(5):
        wt4z[:, dt * 33 + 0] = W4a[0, :, dt]
        wt4z[:, dt * 33 + 32] = W4a[1, :, dt]

    in_maps = []
    for core in range(NCORES):
        bs = slice(core * NB, (core + 1) * NB)
        xsb = xs[bs]; ysb = ys[bs]; xtsb = xts[bs]
        xpoly = np.concatenate(
            [np.stack([x * x, x, np.ones_like(x)]) for x in xsb], axis=1)
        xtpoly = np.concatenate(
            [np.stack([np.ones_like(x), -2.0 * x, x * x]) for x in xtsb], axis=1)
        phip = np.zeros((128, NB * ENC_JT * 33), np.float32)
        for bb in range(NB):
            for jt in range(ENC_JT):
                c0 = (bb * ENC_JT + jt) * 33
                phip[:, c0] = 1.0
                phip[:, c0 + 32] = ysb[bb, jt * 128: jt * 128 + 128]
        in_maps.append({
            "xpoly": xpoly.astype(np.float32),
            "tpoly": tpoly, "tpoly2": tpoly2,
            "xtpoly": xtpoly.astype(np.float32),
            "trow": trow, "phip": phip,
            "wt1": wt1, "wt2": wt2, "wt3": wt3, "wt4": wt4z,
            "b1v": np.asarray(b1, np.float32).reshape(16, 1),
            "b2v": np.asarray(b2, np.float32).reshape(32, 1),
            "b3v": np.asarray(b3, np.float32).reshape(16, 1),
        })

    global _LAST_MAPS
    _LAST_MAPS = in_maps
    import time as _time
    _t0 = _time.time()
    res = bass_utils.run_bass_kernel_spmd(nc, in_maps, core_ids=list(range(NCORES)))
    global _LAST_WALL
    _LAST_WALL = _time.time() - _t0

    mu = np.zeros((B, NT), np.float32)
    sigma = np.zeros((B, NT), np.float32)
    for core in range(NCORES):
        out = res.results[core]
        for bb in range(NB):
            gb = core * NB + bb
            inv = ixt[gb]
            mu[gb, inv] = out["mu_out"][bb]
            sigma[gb, inv] = out["sig_out"][bb]

    Sigma = np.zeros((B, NT, NT), np.float32)
    idx = np.arange(NT)
    Sigma[:, idx, idx] = sigma
    return mu, Sigma


# revision 10
# speedup vs baseline: 1.2148x; 1.2148x over previous
import sys

sys.path.insert(0, "/opt/trn_rl_repo")

import numpy as np

NUM_T = 1024
B, NC, NT = 32, 2048, 1024
NB = 4          # batches per core
NCORES = 8
ENC_JT = 16     # j-tiles of 128 sorted xc points
DEC_G = 4       # xt chunks of 256 sorted xt points

_CACHE = {}


def _build(key, repeat=1):
    (scale_psi, lnos_psi, scale_rho, lnos_rho, b4_0, b4_1,
     enc_W, enc_offs, dec_bands) = key
    from concourse import bass, bacc, tile, mybir

    F32 = mybir.dt.float32
    F32R = mybir.dt.float32r
    AF = mybir.ActivationFunctionType

    nc = bacc.Bacc("TRN2", target_bir_lowering=False, debug=False,
                   enable_asserts=False, num_devices=1)

    di = {}
    for name, shape in [
        ("xpoly", (3, NB * NC)), ("tpoly", (3, NUM_T)), ("tpoly2", (3, NUM_T)),
        ("xtpoly", (3, NB * NT)), ("trow", (1, NUM_T)),
        ("phip", (128, NB * ENC_JT * 33)),
        ("wt1", (3, 5 * 16)), ("wt2", (16, 5 * 32)), ("wt3", (32, 5 * 16)),
        ("wt4", (16, 5 * 33)),
        ("b1v", (16, 1)), ("b2v", (32, 1)), ("b3v", (16, 1)),
    ]:
        di[name] = nc.dram_tensor(name, shape, F32, kind="ExternalInput")
    d_mu = nc.dram_tensor("mu_out", (NB, NT), F32, kind="ExternalOutput")
    d_sg = nc.dram_tensor("sig_out", (NB, NT), F32, kind="ExternalOutput")

    with tile.TileContext(nc) as tc:
        with (
            tc.tile_pool(name="cst", bufs=1) as cst,
            tc.tile_pool(name="wrk", bufs=1) as wrk,
            tc.tile_pool(name="kep", bufs=2) as kep,
            tc.tile_pool(name="psw", bufs=2, space="PSUM") as psw,
            tc.tile_pool(name="psh", bufs=1, space="PSUM") as psh,
            tc.tile_pool(name="psm", bufs=1, space="PSUM") as psm,
        ):
            # ---- persistent constants ----
            c_xpoly = cst.tile([3, NB * NC], F32)
            nc.sync.dma_start(c_xpoly[:], di["xpoly"].ap())
            c_tpoly = cst.tile([3, NUM_T], F32)
            nc.sync.dma_start(c_tpoly[:], di["tpoly"].ap())
            c_tpoly2 = cst.tile([3, NUM_T], F32)
            nc.sync.dma_start(c_tpoly2[:], di["tpoly2"].ap())
            c_xtpoly = cst.tile([3, NB * NT], F32)
            nc.sync.dma_start(c_xtpoly[:], di["xtpoly"].ap())
            c_trow = cst.tile([1, NUM_T], F32)
            nc.sync.dma_start(c_trow[:], di["trow"].ap())
            c_phip = cst.tile([128, NB * ENC_JT * 33], F32R)
            nc.gpsimd.dma_start(c_phip[:], di["phip"].ap())
            c_wt1 = cst.tile([3, 80], F32R); nc.gpsimd.dma_start(c_wt1[:], di["wt1"].ap())
            c_wt2 = cst.tile([16, 160], F32R); nc.gpsimd.dma_start(c_wt2[:], di["wt2"].ap())
            c_wt3 = cst.tile([32, 80], F32R); nc.gpsimd.dma_start(c_wt3[:], di["wt3"].ap())
            c_wt4 = cst.tile([16, 165], F32R); nc.gpsimd.dma_start(c_wt4[:], di["wt4"].ap())
            c_b1 = cst.tile([16, 1], F32); nc.sync.dma_start(c_b1[:], di["b1v"].ap())
            c_b2 = cst.tile([32, 1], F32); nc.sync.dma_start(c_b2[:], di["b2v"].ap())
            c_b3 = cst.tile([16, 1], F32); nc.sync.dma_start(c_b3[:], di["b3v"].ap())

            c_bpsi = cst.tile([128, 1], F32); nc.vector.memset(c_bpsi[:], lnos_psi)
            c_brho = cst.tile([128, 1], F32); nc.vector.memset(c_brho[:], lnos_rho)
            c_b41 = cst.tile([33, 1], F32); nc.vector.memset(c_b41[:], b4_1)
            c_one = cst.tile([33, 1], F32); nc.vector.memset(c_one[:], 1.0)
            z_l33 = cst.tile([1, 33], F32R); nc.vector.memset(z_l33[:].bitcast(F32), 0.0)
            cX1 = cst.tile([3, 1028], F32R, name="cX1")
            nc.vector.memset(cX1[:].bitcast(F32), 0.0)
            nc.gpsimd.dma_start(cX1[0:1, 2:1026], c_trow[:])
            cX2 = cst.tile([16, 1028], F32R, name="cX2")
            nc.vector.memset(cX2[:].bitcast(F32), 0.0)
            cX3 = cst.tile([32, 1028], F32R, name="cX3")
            nc.vector.memset(cX3[:].bitcast(F32), 0.0)
            cX4 = cst.tile([16, 1028], F32R, name="cX4")
            nc.vector.memset(cX4[:].bitcast(F32), 0.0)
            z_r = cst.tile([1, 512], F32R); nc.vector.memset(z_r[:].bitcast(F32), 0.0)

            def _body():
              for b in range(NB):
                # ================= encoder =================
                ph = psh.tile([33, NUM_T], F32, name="ph", tag="ph")
                # zero-fill ph with has_written set, via zero matmuls
                nc.tensor.matmul(ph[:, 0:512], z_l33[:], z_r[:], start=True, stop=False)
                nc.tensor.matmul(ph[:, 512:1024], z_l33[:], z_r[:], start=True, stop=False)

                # last h-mm piece per bank, to set stop flags
                pieces_all = []
                for jt in range(ENC_JT):
                    o = enc_offs[jt]
                    ps = []
                    if o < 512:
                        ps.append((o, min(o + enc_W, 512)))
                    if o + enc_W > 512:
                        ps.append((max(o, 512), o + enc_W))
                    pieces_all.append(ps)
                last_in_bank = {0: None, 1: None}
                for jt in range(ENC_JT):
                    for pi, (lo, hi) in enumerate(pieces_all[jt]):
                        last_in_bank[0 if lo < 512 else 1] = (jt, pi)

                GRP = 3  # j-tiles per psum tile
                for g0 in range(0, ENC_JT, GRP):
                    jts = list(range(g0, min(g0 + GRP, ENC_JT)))
                    pd = psw.tile([128, NUM_T], F32, name="pd", tag="pw")
                    for idx, jt in enumerate(jts):
                        o = enc_offs[jt]
                        s0 = enc_W * idx
                        # split the output slot at psum bank (512) boundaries
                        cuts = [s0, s0 + enc_W]
                        for edge in (512, 1024):
                            if s0 < edge < s0 + enc_W:
                                cuts.insert(-1, edge)
                        for lo, hi in zip(cuts[:-1], cuts[1:]):
                            nc.tensor.matmul(
                                pd[:, lo:hi],
                                c_xpoly[:, b * NC + jt * 128: b * NC + jt * 128 + 128],
                                c_tpoly[:, o + (lo - s0): o + (hi - s0)],
                                start=True, stop=True)
                    ke = kep.tile([128, NUM_T], F32R, name="ke", tag="ke")
                    nc.scalar.activation(
                        ke[:, 0: enc_W * len(jts)], pd[:, 0: enc_W * len(jts)],
                        AF.Exp, bias=c_bpsi[:], scale=scale_psi)
                    for idx, jt in enumerate(jts):
                        o = enc_offs[jt]
                        pc = (b * ENC_JT + jt) * 33
                        for pi, (lo, hi) in enumerate(pieces_all[jt]):
                            stop = (jt, pi) == last_in_bank[0 if lo < 512 else 1]
                            nc.tensor.matmul(
                                ph[:, lo:hi],
                                c_phip[:, pc: pc + 33],
                                ke[:, enc_W * idx + (lo - o): enc_W * idx + (hi - o)],
                                start=False, stop=stop)

                # ---- h features -> conv input X1 ----
                hs0 = wrk.tile([1, NUM_T], F32, name="hs0", tag="hs0")
                nc.vector.tensor_copy(hs0[:], ph[0:1, :])
                hs1 = wrk.tile([33, NUM_T], F32, name="hs1", tag="hs1")
                nc.vector.tensor_copy(hs1[32:33, :], ph[32:33, :])
                tA = wrk.tile([1, NUM_T], F32, name="tA", tag="tA")
                nc.vector.tensor_scalar_add(tA[:], hs0[:], 1e-8)
                tR = wrk.tile([1, NUM_T], F32, name="tR", tag="tR")
                nc.vector.reciprocal(tR[:], tA[:])
                tR32 = wrk.tile([33, NUM_T], F32, name="tR32", tag="tR32")
                nc.sync.dma_start(tR32[32:33, :], tR[:])
                tM = wrk.tile([33, NUM_T], F32, name="tM", tag="tM")
                nc.vector.tensor_mul(tM[32:33, :], hs1[32:33, :], tR32[32:33, :])

                X1 = cX1
                nc.gpsimd.dma_start(X1[1:2, 2:1026], hs0[:])
                nc.gpsimd.dma_start(X1[2:3, 2:1026], tM[32:33, :])

                # ================= conv tower =================
                def conv_layer(xin, wt, otile, bias_ap, xo, oname):
                    pc = psw.tile([max(otile, 33), NUM_T], F32, name="pc" + oname, tag="pw")
                    for ch in (0, 512):
                        for dt in range(5):
                            nc.tensor.matmul(
                                pc[0:otile, ch: ch + 512],
                                wt[:, dt * otile: (dt + 1) * otile],
                                xin[:, dt + ch: dt + ch + 512],
                                start=(dt == 0), stop=(dt == 4))
                    nc.scalar.activation(xo[:, 2:1026], pc[0:otile, :], AF.Relu,
                                         bias=bias_ap, scale=1.0)
                    return xo

                X2 = conv_layer(X1, c_wt1, 16, c_b1[:], cX2, "X2")
                X3 = conv_layer(X2, c_wt2, 32, c_b2[:], cX3, "X3")
                X4 = conv_layer(X3, c_wt3, 16, c_b3[:], cX4, "X4")

                p4 = psw.tile([33, NUM_T], F32, name="p4", tag="pw")
                for ch in (0, 512):
                    for dt in range(5):
                        nc.tensor.matmul(
                            p4[:, ch: ch + 512],
                            c_wt4[:, dt * 33: (dt + 1) * 33],
                            X4[:, dt + ch: dt + ch + 512],
                            start=(dt == 0), stop=(dt == 4))

                S0 = wrk.tile([1, NUM_T], F32, name="S0", tag="S0")
                nc.vector.tensor_scalar_add(S0[:], p4[0:1, :], b4_0)
                E32 = wrk.tile([33, NUM_T], F32, name="E32", tag="E32")
                nc.scalar.activation(E32[32:33, :], p4[32:33, :], AF.Exp,
                                     bias=c_b41[32:33, :], scale=1.0)
                SP = wrk.tile([33, NUM_T], F32, name="SP", tag="SP")
                nc.scalar.activation(SP[32:33, :], E32[32:33, :], AF.Ln,
                                     bias=c_one[32:33, :], scale=1.0)

                FSC = wrk.tile([32, NUM_T], F32, name="FSC", tag="FSC")
                nc.vector.memset(FSC[:], 0.0)
                nc.sync.dma_start(FSC[0:1, :], S0[:])
                nc.sync.dma_start(FSC[1:2, :], SP[32:33, :])
                FT32 = wrk.tile([128, 256], F32, name="FT32", tag="FT32")
                for k in range(8):
                    for m in range(4):
                        nc.vector.transpose(
                            FT32[32 * m: 32 * m + 32, 32 * k: 32 * k + 32],
                            FSC[:, 128 * k + 32 * m: 128 * k + 32 * m + 32])
                FDW = wrk.tile([128, 256], F32R, name="FDW", tag="FDW")
                nc.gpsimd.dma_start(FDW[:], FT32[:])

                # ================= decoder =================
                pmu = psm.tile([2, NUM_T], F32, name="pmu", tag="pmu")
                for g in range(DEC_G):
                    u0, kk = dec_bands[g]
                    pd2 = psw.tile([128, NUM_T], F32, name="pd2", tag="pw")
                    for s in range(kk):
                        nc.tensor.matmul(
                            pd2[:, 256 * s: 256 * s + 256],
                            c_tpoly2[:, 128 * (u0 + s): 128 * (u0 + s) + 128],
                            c_xtpoly[:, b * NT + 256 * g: b * NT + 256 * g + 256],
                            start=True, stop=True)
                    k2 = kep.tile([128, NUM_T], F32R, name="k2", tag="k2")
                    nc.scalar.activation(k2[:, 0: 256 * kk], pd2[:, 0: 256 * kk],
                                         AF.Exp, bias=c_brho[:], scale=scale_rho)
                    for s in range(kk):
                        nc.tensor.matmul(
                            pmu[:, 256 * g: 256 * g + 256],
                            FDW[:, 32 * (u0 + s): 32 * (u0 + s) + 2],
                            k2[:, 256 * s: 256 * s + 256],
                            start=(s == 0), stop=(s == kk - 1))

                MU = wrk.tile([2, NUM_T], F32, name="MU", tag="MU")
                nc.vector.tensor_copy(MU[:], pmu[:])
                nc.sync.dma_start(d_mu.ap()[b: b + 1, :], MU[0:1, :])
                nc.sync.dma_start(d_sg.ap()[b: b + 1, :], MU[1:2, :])

            if repeat > 1:
                with tc.For_i(0, repeat, 1):
                    _body()
            else:
                _body()

    nc.compile()
    return nc


def kernel(xc, yc, xt, ls_psi, os_psi, ls_rho, os_rho,
           W1, b1, W2, b2, W3, b3, W4, b4):
    from concourse import bass_utils

    xc = np.asarray(xc, np.float32); yc = np.asarray(yc, np.float32)
    xt = np.asarray(xt, np.float32)
    ls_p = float(np.asarray(ls_psi).reshape(-1)[0])
    os_p = float(np.asarray(os_psi).reshape(-1)[0])
    ls_r = float(np.asarray(ls_rho).reshape(-1)[0])
    os_r = float(np.asarray(os_rho).reshape(-1)[0])

    xcf = xc[..., 0]; ycf = yc[..., 0]; xtf = xt[..., 0]
    lower = min(xcf.min(), xtf.min())
    upper = max(xcf.max(), xtf.max())
    t = np.linspace(lower, upper, NUM_T, dtype=np.float64).astype(np.float32)
    step = (upper - lower) / (NUM_T - 1)

    # sort per batch
    ixc = np.argsort(xcf, axis=1)
    xs = np.take_along_axis(xcf, ixc, axis=1)
    ys = np.take_along_axis(ycf, ixc, axis=1)
    ixt = np.argsort(xtf, axis=1)
    xts = np.take_along_axis(xtf, ixt, axis=1)

    # ---- global static windows (shared across cores) ----
    tau_e = 7.3 * ls_p
    tau_d = 7.3 * ls_r
    enc_offs = []
    enc_W = 288
    spans = []
    for jt in range(ENC_JT):
        lo = xs[:, jt * 128].min() - tau_e
        hi = xs[:, jt * 128 + 127].max() + tau_e
        spans.append(hi - lo)
    wmax = int(np.ceil(max(spans) / step)) + 4
    enc_W = max(288, ((wmax + 31) // 32) * 32)
    enc_W = min(enc_W, 1024)
    for jt in range(ENC_JT):
        lo = xs[:, jt * 128].min() - tau_e
        hi = xs[:, jt * 128 + 127].max() + tau_e
        mid = 0.5 * (lo + hi)
        o = int(round((mid - lower) / step)) - enc_W // 2
        o = max(0, min(NUM_T - enc_W, o))
        o = (o // 2) * 2
        # verify coverage
        assert o * step + lower <= lo + 1e-4 or o == 0
        assert (o + enc_W - 1) * step + lower >= hi - 1e-4 or o == NUM_T - enc_W
        enc_offs.append(o)

    dec_bands = []
    for g in range(DEC_G):
        lo = xts[:, g * 256].min() - tau_d
        hi = xts[:, g * 256 + 255].max() + tau_d
        u0 = int(np.floor((lo - lower) / step / 128.0))
        u1 = int(np.ceil((hi - lower) / step / 128.0))
        u0 = max(0, u0); u1 = min(8, u1); u1 = max(u1, u0 + 1)
        if u1 - u0 > 4:  # cap at 4 slabs (one psum bank)
            u0, u1 = u0, u0 + 4
        dec_bands.append((u0, u1 - u0))

    key = (float(-0.5 / (ls_p * ls_p)), float(np.log(os_p)),
           float(-0.5 / (ls_r * ls_r)), float(np.log(os_r)),
           float(np.asarray(b4).reshape(-1)[0]), float(np.asarray(b4).reshape(-1)[1]),
           enc_W, tuple(enc_offs), tuple(dec_bands))
    if key not in _CACHE:
        _CACHE[key] = _build(key)
    nc = _CACHE[key]
    global _LAST_KEY
    _LAST_KEY = key

    # ---- per-core input maps ----
    ones_t = np.ones_like(t)
    tpoly = np.stack([ones_t, -2.0 * t, t * t]).astype(np.float32)
    tpoly2 = np.stack([t * t, t, ones_t]).astype(np.float32)
    trow = t.reshape(1, -1)
    wt1 = np.transpose(np.asarray(W1, np.float32), (1, 2, 0)).reshape(3, 5 * 16, order="F")
    # careful: want wt[c, dt*O + o] = W[o, c, dt]
    def pack_w(W, O, C):
        out = np.zeros((C, 5 * O), np.float32)
        for dt in range(5):
            out[:, dt * O: (dt + 1) * O] = np.asarray(W, np.float32)[:, :, dt].T
        return out
    wt1 = pack_w(W1, 16, 3)
    wt2 = pack_w(W2, 32, 16)
    wt3 = pack_w(W3, 16, 32)
    wt4z = np.zeros((16, 5 * 33), np.float32)
    W4a = np.asarray(W4, np.float32)
    for dt in range(5):
        wt4z[:, dt * 33 + 0] = W4a[0, :, dt]
        wt4z[:, dt * 33 + 32] = W4a[1, :, dt]

    in_maps = []
    for core in range(NCORES):
        bs = slice(core * NB, (core + 1) * NB)
        xsb = xs[bs]; ysb = ys[bs]; xtsb = xts[bs]
        xpoly = np.concatenate(
            [np.stack([x * x, x, np.ones_like(x)]) for x in xsb], axis=1)
        xtpoly = np.concatenate(
            [np.stack([np.ones_like(x), -2.0 * x, x * x]) for x in xtsb], axis=1)
        phip = np.zeros((128, NB * ENC_JT * 33), np.float32)
        for bb in range(NB):
            for jt in range(ENC_JT):
                c0 = (bb * ENC_JT + jt) * 33
                phip[:, c0] = 1.0
                phip[:, c0 + 32] = ysb[bb, jt * 128: jt * 128 + 128]
        in_maps.append({
            "xpoly": xpoly.astype(np.float32),
            "tpoly": tpoly, "tpoly2": tpoly2,
            "xtpoly": xtpoly.astype(np.float32),
            "trow": trow, "phip": phip,
            "wt1": wt1, "wt2": wt2, "wt3": wt3, "wt4": wt4z,
            "b1v": np.asarray(b1, np.float32).reshape(16, 1),
            "b2v": np.asarray(b2, np.float32).reshape(32, 1),
            "b3v": np.asarray(b3, np.float32).reshape(16, 1),
        })

    global _LAST_MAPS
    _LAST_MAPS = in_maps
    import time as _time
    _t0 = _time.time()
    res = bass_utils.run_bass_kernel_spmd(nc, in_maps, core_ids=list(range(NCORES)))
    global _LAST_WALL
    _LAST_WALL = _time.time() - _t0

    mu = np.zeros((B, NT), np.float32)
    sigma = np.zeros((B, NT), np.float32)
    for core in range(NCORES):
        out = res.results[core]
        for bb in range(NB):
            gb = core * NB + bb
            inv = ixt[gb]
            mu[gb, inv] = out["mu_out"][bb]
            sigma[gb, inv] = out["sig_out"][bb]

    Sigma = np.zeros((B, NT, NT), np.float32)
    idx = np.arange(NT)
    Sigma[:, idx, idx] = sigma
    return mu, Sigma


# revision 11
# speedup vs baseline: 1.7958x; 1.4783x over previous
import sys

sys.path.insert(0, "/opt/trn_rl_repo")

import numpy as np

NUM_T = 1024
B, NC, NT = 32, 2048, 1024
NB = 4          # batches per core
NCORES = 8
ENC_JT = 16     # j-tiles of 128 sorted xc points
DEC_G = 4       # xt chunks of 256 sorted xt points

_CACHE = {}


def _build(key, repeat=1):
    (scale_psi, lnos_psi, scale_rho, lnos_rho, b4_0, b4_1,
     enc_W, enc_offs, dec_bands) = key
    from concourse import bass, bacc, tile, mybir

    F32 = mybir.dt.float32
    F32R = mybir.dt.float32r
    AF = mybir.ActivationFunctionType

    nc = bacc.Bacc("TRN2", target_bir_lowering=False, debug=False,
                   enable_asserts=False, num_devices=1)

    di = {}
    for name, shape in [
        ("xpoly", (3, NB * NC)), ("tpoly", (3, NUM_T)), ("tpoly2", (3, NUM_T)),
        ("xtpoly", (3, NB * NT)), ("trow", (1, NUM_T)),
        ("phip", (128, NB * ENC_JT * 33)),
        ("wt1", (3, 5 * 16)), ("wt2", (16, 5 * 32)), ("wt3", (32, 5 * 16)),
        ("wt4", (16, 5 * 33)),
        ("b1v", (16, 1)), ("b2v", (32, 1)), ("b3v", (16, 1)),
    ]:
        di[name] = nc.dram_tensor(name, shape, F32, kind="ExternalInput")
    d_mu = nc.dram_tensor("mu_out", (NB, NT), F32, kind="ExternalOutput")
    d_sg = nc.dram_tensor("sig_out", (NB, NT), F32, kind="ExternalOutput")

    with tile.TileContext(nc) as tc:
        with (
            tc.tile_pool(name="cst", bufs=1) as cst,
            tc.tile_pool(name="wrk", bufs=1) as wrk,
            tc.tile_pool(name="kep", bufs=2) as kep,
            tc.tile_pool(name="psw", bufs=2, space="PSUM") as psw,
            tc.tile_pool(name="psh", bufs=1, space="PSUM") as psh,
            tc.tile_pool(name="psm", bufs=1, space="PSUM") as psm,
        ):
            # ---- persistent constants ----
            c_xpoly = cst.tile([3, NB * NC], F32)
            nc.sync.dma_start(c_xpoly[:], di["xpoly"].ap())
            c_tpoly = cst.tile([3, NUM_T], F32)
            nc.sync.dma_start(c_tpoly[:], di["tpoly"].ap())
            c_tpoly2 = cst.tile([3, NUM_T], F32)
            nc.sync.dma_start(c_tpoly2[:], di["tpoly2"].ap())
            c_xtpoly = cst.tile([3, NB * NT], F32)
            nc.sync.dma_start(c_xtpoly[:], di["xtpoly"].ap())
            c_trow = cst.tile([1, NUM_T], F32)
            nc.sync.dma_start(c_trow[:], di["trow"].ap())
            c_phip = cst.tile([128, NB * ENC_JT * 33], F32R)
            nc.gpsimd.dma_start(c_phip[:], di["phip"].ap())
            c_wt1 = cst.tile([3, 80], F32R); nc.gpsimd.dma_start(c_wt1[:], di["wt1"].ap())
            c_wt2 = cst.tile([16, 160], F32R); nc.gpsimd.dma_start(c_wt2[:], di["wt2"].ap())
            c_wt3 = cst.tile([32, 80], F32R); nc.gpsimd.dma_start(c_wt3[:], di["wt3"].ap())
            c_wt4 = cst.tile([16, 165], F32R); nc.gpsimd.dma_start(c_wt4[:], di["wt4"].ap())
            c_b1 = cst.tile([16, 1], F32); nc.sync.dma_start(c_b1[:], di["b1v"].ap())
            c_b2 = cst.tile([32, 1], F32); nc.sync.dma_start(c_b2[:], di["b2v"].ap())
            c_b3 = cst.tile([16, 1], F32); nc.sync.dma_start(c_b3[:], di["b3v"].ap())

            c_bpsi = cst.tile([128, 1], F32); nc.vector.memset(c_bpsi[:], lnos_psi)
            c_brho = cst.tile([128, 1], F32); nc.vector.memset(c_brho[:], lnos_rho)
            c_b41 = cst.tile([33, 1], F32); nc.vector.memset(c_b41[:], b4_1)
            c_one = cst.tile([33, 1], F32); nc.vector.memset(c_one[:], 1.0)
            z_l33 = cst.tile([1, 33], F32R); nc.vector.memset(z_l33[:].bitcast(F32), 0.0)
            cX1 = cst.tile([3, 1028], F32R, name="cX1")
            nc.vector.memset(cX1[:].bitcast(F32), 0.0)
            nc.gpsimd.dma_start(cX1[0:1, 2:1026], c_trow[:])
            cX2 = cst.tile([16, 1028], F32R, name="cX2")
            nc.vector.memset(cX2[:].bitcast(F32), 0.0)
            cX3 = cst.tile([32, 1028], F32R, name="cX3")
            nc.vector.memset(cX3[:].bitcast(F32), 0.0)
            cX4 = cst.tile([16, 1028], F32R, name="cX4")
            nc.vector.memset(cX4[:].bitcast(F32), 0.0)
            z_r = cst.tile([1, 512], F32R); nc.vector.memset(z_r[:].bitcast(F32), 0.0)

            def _body():
              for b in range(NB):
                # ================= encoder =================
                ph = psh.tile([33, NUM_T], F32, name="ph", tag="ph")
                # zero-fill ph with has_written set, via zero matmuls
                nc.tensor.matmul(ph[:, 0:512], z_l33[:], z_r[:], start=True, stop=False)
                nc.tensor.matmul(ph[:, 512:1024], z_l33[:], z_r[:], start=True, stop=False)

                # last h-mm piece per bank, to set stop flags
                pieces_all = []
                for jt in range(ENC_JT):
                    o = enc_offs[jt]
                    ps = []
                    if o < 512:
                        ps.append((o, min(o + enc_W, 512)))
                    if o + enc_W > 512:
                        ps.append((max(o, 512), o + enc_W))
                    pieces_all.append(ps)
                last_in_bank = {0: None, 1: None}
                for jt in range(ENC_JT):
                    for pi, (lo, hi) in enumerate(pieces_all[jt]):
                        last_in_bank[0 if lo < 512 else 1] = (jt, pi)

                GRP = 3  # j-tiles per psum tile
                for g0 in range(0, ENC_JT, GRP):
                    jts = list(range(g0, min(g0 + GRP, ENC_JT)))
                    pd = psw.tile([128, NUM_T], F32, name="pd", tag="pw")
                    for idx, jt in enumerate(jts):
                        o = enc_offs[jt]
                        s0 = enc_W * idx
                        # split the output slot at psum bank (512) boundaries
                        cuts = [s0, s0 + enc_W]
                        for edge in (512, 1024):
                            if s0 < edge < s0 + enc_W:
                                cuts.insert(-1, edge)
                        for lo, hi in zip(cuts[:-1], cuts[1:]):
                            nc.tensor.matmul(
                                pd[:, lo:hi],
                                c_xpoly[:, b * NC + jt * 128: b * NC + jt * 128 + 128],
                                c_tpoly[:, o + (lo - s0): o + (hi - s0)],
                                start=True, stop=True)
                    ke = kep.tile([128, NUM_T], F32R, name="ke", tag="ke")
                    nc.scalar.activation(
                        ke[:, 0: enc_W * len(jts)], pd[:, 0: enc_W * len(jts)],
                        AF.Exp, bias=c_bpsi[:], scale=scale_psi)
                    for idx, jt in enumerate(jts):
                        o = enc_offs[jt]
                        pc = (b * ENC_JT + jt) * 33
                        for pi, (lo, hi) in enumerate(pieces_all[jt]):
                            stop = (jt, pi) == last_in_bank[0 if lo < 512 else 1]
                            nc.tensor.matmul(
                                ph[:, lo:hi],
                                c_phip[:, pc: pc + 33],
                                ke[:, enc_W * idx + (lo - o): enc_W * idx + (hi - o)],
                                start=False, stop=stop)

                # ---- h features -> conv input X1 ----
                hs0 = wrk.tile([1, NUM_T], F32, name="hs0", tag="hs0")
                nc.vector.tensor_copy(hs0[:], ph[0:1, :])
                hs1 = wrk.tile([33, NUM_T], F32, name="hs1", tag="hs1")
                nc.vector.tensor_copy(hs1[32:33, :], ph[32:33, :])
                tA = wrk.tile([1, NUM_T], F32, name="tA", tag="tA")
                nc.vector.tensor_scalar_add(tA[:], hs0[:], 1e-8)
                tR = wrk.tile([1, NUM_T], F32, name="tR", tag="tR")
                nc.vector.reciprocal(tR[:], tA[:])
                tR32 = wrk.tile([33, NUM_T], F32, name="tR32", tag="tR32")
                nc.sync.dma_start(tR32[32:33, :], tR[:])
                tM = wrk.tile([33, NUM_T], F32, name="tM", tag="tM")
                nc.vector.tensor_mul(tM[32:33, :], hs1[32:33, :], tR32[32:33, :])

                X1 = cX1
                nc.gpsimd.dma_start(X1[1:2, 2:1026], hs0[:])
                nc.gpsimd.dma_start(X1[2:3, 2:1026], tM[32:33, :])

                # ================= conv tower =================
                def conv_layer(xin, wt, otile, bias_ap, xo, oname):
                    pc = psw.tile([max(otile, 33), NUM_T], F32, name="pc" + oname, tag="pw")
                    for ch in (0, 512):
                        for dt in range(5):
                            nc.tensor.matmul(
                                pc[0:otile, ch: ch + 512],
                                wt[:, dt * otile: (dt + 1) * otile],
                                xin[:, dt + ch: dt + ch + 512],
                                start=(dt == 0), stop=(dt == 4))
                    nc.scalar.activation(xo[:, 2:1026], pc[0:otile, :], AF.Relu,
                                         bias=bias_ap, scale=1.0)
                    return xo

                X2 = conv_layer(X1, c_wt1, 16, c_b1[:], cX2, "X2")
                X3 = conv_layer(X2, c_wt2, 32, c_b2[:], cX3, "X3")
                X4 = conv_layer(X3, c_wt3, 16, c_b3[:], cX4, "X4")

                p4 = psw.tile([33, NUM_T], F32, name="p4", tag="pw")
                for ch in (0, 512):
                    for dt in range(5):
                        nc.tensor.matmul(
                            p4[:, ch: ch + 512],
                            c_wt4[:, dt * 33: (dt + 1) * 33],
                            X4[:, dt + ch: dt + ch + 512],
                            start=(dt == 0), stop=(dt == 4))

                S0 = wrk.tile([1, NUM_T], F32, name="S0", tag="S0")
                nc.vector.tensor_scalar_add(S0[:], p4[0:1, :], b4_0)
                E32 = wrk.tile([33, NUM_T], F32, name="E32", tag="E32")
                nc.scalar.activation(E32[32:33, :], p4[32:33, :], AF.Exp,
                                     bias=c_b41[32:33, :], scale=1.0)
                SP = wrk.tile([33, NUM_T], F32, name="SP", tag="SP")
                nc.scalar.activation(SP[32:33, :], E32[32:33, :], AF.Ln,
                                     bias=c_one[32:33, :], scale=1.0)

                FSC = wrk.tile([32, NUM_T], F32, name="FSC", tag="FSC")
                nc.vector.memset(FSC[:], 0.0)
                nc.sync.dma_start(FSC[0:1, :], S0[:])
                nc.sync.dma_start(FSC[1:2, :], SP[32:33, :])
                FT32 = wrk.tile([128, 256], F32, name="FT32", tag="FT32")
                for k in range(8):
                    for m in range(4):
                        nc.vector.transpose(
                            FT32[32 * m: 32 * m + 32, 32 * k: 32 * k + 32],
                            FSC[:, 128 * k + 32 * m: 128 * k + 32 * m + 32])
                FDW = wrk.tile([128, 256], F32R, name="FDW", tag="FDW")
                nc.gpsimd.dma_start(FDW[:], FT32[:])

                # ================= decoder =================
                pmu = psm.tile([2, NUM_T], F32, name="pmu", tag="pmu")
                for g in range(DEC_G):
                    u0, kk = dec_bands[g]
                    pd2 = psw.tile([128, NUM_T], F32, name="pd2", tag="pw")
                    for s in range(kk):
                        nc.tensor.matmul(
                            pd2[:, 256 * s: 256 * s + 256],
                            c_tpoly2[:, 128 * (u0 + s): 128 * (u0 + s) + 128],
                            c_xtpoly[:, b * NT + 256 * g: b * NT + 256 * g + 256],
                            start=True, stop=True)
                    k2 = kep.tile([128, NUM_T], F32R, name="k2", tag="k2")
                    nc.scalar.activation(k2[:, 0: 256 * kk], pd2[:, 0: 256 * kk],
                                         AF.Exp, bias=c_brho[:], scale=scale_rho)
                    for s in range(kk):
                        nc.tensor.matmul(
                            pmu[:, 256 * g: 256 * g + 256],
                            FDW[:, 32 * (u0 + s): 32 * (u0 + s) + 2],
                            k2[:, 256 * s: 256 * s + 256],
                            start=(s == 0), stop=(s == kk - 1))

                MU = wrk.tile([2, NUM_T], F32, name="MU", tag="MU")
                nc.vector.tensor_copy(MU[:], pmu[:])
                nc.sync.dma_start(d_mu.ap()[b: b + 1, :], MU[0:1, :])
                nc.sync.dma_start(d_sg.ap()[b: b + 1, :], MU[1:2, :])

            if repeat > 1:
                with tc.For_i(0, repeat, 1):
                    _body()
            else:
                _body()

    nc.compile()
    return nc


def kernel(xc, yc, xt, ls_psi, os_psi, ls_rho, os_rho,
           W1, b1, W2, b2, W3, b3, W4, b4):
    from concourse import bass_utils

    xc = np.asarray(xc, np.float32); yc = np.asarray(yc, np.float32)
    xt = np.asarray(xt, np.float32)
    ls_p = float(np.asarray(ls_psi).reshape(-1)[0])
    os_p = float(np.asarray(os_psi).reshape(-1)[0])
    ls_r = float(np.asarray(ls_rho).reshape(-1)[0])
    os_r = float(np.asarray(os_rho).reshape(-1)[0])

    xcf = xc[..., 0]; ycf = yc[..., 0]; xtf = xt[..., 0]
    lower = min(xcf.min(), xtf.min())
    upper = max(xcf.max(), xtf.max())
    t = np.linspace(lower, upper, NUM_T, dtype=np.float64).astype(np.float32)
    step = (upper - lower) / (NUM_T - 1)

    # sort per batch
    ixc = np.argsort(xcf, axis=1)
    xs = np.take_along_axis(xcf, ixc, axis=1)
    ys = np.take_along_axis(ycf, ixc, axis=1)
    ixt = np.argsort(xtf, axis=1)
    xts = np.take_along_axis(xtf, ixt, axis=1)

    # ---- global static windows (shared across cores) ----
    tau_e = 6.5 * ls_p
    tau_d = 6.5 * ls_r
    enc_offs = []
    enc_W = 288
    spans = []
    for jt in range(ENC_JT):
        lo = xs[:, jt * 128].min() - tau_e
        hi = xs[:, jt * 128 + 127].max() + tau_e
        spans.append(hi - lo)
    wmax = int(np.ceil(max(spans) / step)) + 4
    enc_W = max(256, ((wmax + 31) // 32) * 32)
    enc_W = min(enc_W, 1024)
    for jt in range(ENC_JT):
        lo = xs[:, jt * 128].min() - tau_e
        hi = xs[:, jt * 128 + 127].max() + tau_e
        mid = 0.5 * (lo + hi)
        o = int(round((mid - lower) / step)) - enc_W // 2
        o = max(0, min(NUM_T - enc_W, o))
        o = (o // 2) * 2
        # verify coverage
        assert o * step + lower <= lo + 1e-4 or o == 0
        assert (o + enc_W - 1) * step + lower >= hi - 1e-4 or o == NUM_T - enc_W
        enc_offs.append(o)

    dec_bands = []
    for g in range(DEC_G):
        lo = xts[:, g * 256].min() - tau_d
        hi = xts[:, g * 256 + 255].max() + tau_d
        u0 = int(np.floor((lo - lower) / step / 128.0))
        u1 = int(np.ceil((hi - lower) / step / 128.0))
        u0 = max(0, u0); u1 = min(8, u1); u1 = max(u1, u0 + 1)
        if u1 - u0 > 4:  # cap at 4 slabs (one psum bank)
            u0, u1 = u0, u0 + 4
        dec_bands.append((u0, u1 - u0))

    key = (float(-0.5 / (ls_p * ls_p)), float(np.log(os_p)),
           float(-0.5 / (ls_r * ls_r)), float(np.log(os_r)),
           float(np.asarray(b4).reshape(-1)[0]), float(np.asarray(b4).reshape(-1)[1]),
           enc_W, tuple(enc_offs), tuple(dec_bands))
    if key not in _CACHE:
        _CACHE[key] = _build(key)
    nc = _CACHE[key]
    global _LAST_KEY
    _LAST_KEY = key

    # ---- per-core input maps ----
    ones_t = np.ones_like(t)
    tpoly = np.stack([ones_t, -2.0 * t, t * t]).astype(np.float32)
    tpoly2 = np.stack([t * t, t, ones_t]).astype(np.float32)
    trow = t.reshape(1, -1)
    wt1 = np.transpose(np.asarray(W1, np.float32), (1, 2, 0)).reshape(3, 5 * 16, order="F")
    # careful: want wt[c, dt*O + o] = W[o, c, dt]
    def pack_w(W, O, C):
        out = np.zeros((C, 5 * O), np.float32)
        for dt in range(5):
            out[:, dt * O: (dt + 1) * O] = np.asarray(W, np.float32)[:, :, dt].T
        return out
    wt1 = pack_w(W1, 16, 3)
    wt2 = pack_w(W2, 32, 16)
    wt3 = pack_w(W3, 16, 32)
    wt4z = np.zeros((16, 5 * 33), np.float32)
    W4a = np.asarray(W4, np.float32)
    for dt in range(5):
        wt4z[:, dt * 33 + 0] = W4a[0, :, dt]
        wt4z[:, dt * 33 + 32] = W4a[1, :, dt]

    in_maps = []
    for core in range(NCORES):
        bs = slice(core * NB, (core + 1) * NB)
        xsb = xs[bs]; ysb = ys[bs]; xtsb = xts[bs]
        xpoly = np.concatenate(
            [np.stack([x * x, x, np.ones_like(x)]) for x in xsb], axis=1)
        xtpoly = np.concatenate(
            [np.stack([np.ones_like(x), -2.0 * x, x * x]) for x in xtsb], axis=1)
        phip = np.zeros((128, NB * ENC_JT * 33), np.float32)
        for bb in range(NB):
            for jt in range(ENC_JT):
                c0 = (bb * ENC_JT + jt) * 33
                phip[:, c0] = 1.0
                phip[:, c0 + 32] = ysb[bb, jt * 128: jt * 128 + 128]
        in_maps.append({
            "xpoly": xpoly.astype(np.float32),
            "tpoly": tpoly, "tpoly2": tpoly2,
            "xtpoly": xtpoly.astype(np.float32),
            "trow": trow, "phip": phip,
            "wt1": wt1, "wt2": wt2, "wt3": wt3, "wt4": wt4z,
            "b1v": np.asarray(b1, np.float32).reshape(16, 1),
            "b2v": np.asarray(b2, np.float32).reshape(32, 1),
            "b3v": np.asarray(b3, np.float32).reshape(16, 1),
        })

    global _LAST_MAPS
    _LAST_MAPS = in_maps
    import time as _time
    _t0 = _time.time()
    res = bass_utils.run_bass_kernel_spmd(nc, in_maps, core_ids=list(range(NCORES)))
    global _LAST_WALL
    _LAST_WALL = _time.time() - _t0

    mu = np.zeros((B, NT), np.float32)
    sigma = np.zeros((B, NT), np.float32)
    for core in range(NCORES):
        out = res.results[core]
        for bb in range(NB):
            gb = core * NB + bb
            inv = ixt[gb]
            mu[gb, inv] = out["mu_out"][bb]
            sigma[gb, inv] = out["sig_out"][bb]

    Sigma = np.zeros((B, NT, NT), np.float32)
    idx = np.arange(NT)
    Sigma[:, idx, idx] = sigma
    return mu, Sigma
